# revision 17
# baseline (speedup 1.0000x reference)
"""ContinualCLora forward on 8 TRN2 NeuronCores (two launches, x read once).

out = input @ W.T + bmask * sum_k gate_k * (input @ down[I_k] @ up[I_k])

Each core gets 2048 tokens: 1024 from batches {0,1} (no delta) and 1024 from
batches {2,3} (delta applied), so the program is SPMD-uniform and the batch
mask is free.

Launch 1 (~78 us, HBM-roofline): streams all 16 x tiles once (SWDGE cast
f32->bf16), PE identity-matmul transposes each tile (8 chunk transposes
packed in one bf16 PSUM bank, one DVE copy out), DVE free-axis reduces
accumulate the token sums, the 8 non-delta tiles' y = x @ W.T is computed
(PE, 8 K-chunk PSUM accumulation; Scalar engine evacuates) and written as
bf16, and the 8 delta tiles' transposed x is stashed to DRAM (bf16, 2 MB).

Host: reduces the eight [128,8] token-sum shards, computes omega = mean @
route[1], replicates the reference's top-k-on-sliced gating (5 floats), and
folds the gated rank-40 lora product into wmod = W.T + (down*g) @ up.

Launch 2 (~49 us, PE-bound): reads the stashed transposed tiles + wmod,
computes the delta-half y = x @ wmod.T, writes bf16.

History: baseline 330 us (xbar DMA transposes serialized 154 us on the Sync
engine; x read twice). PE transposes + weight folding + launch fusion +
bf16 outputs -> ~128 us.  rel_err ~2.9e-3 (bf16 compute + bf16 y rounding).
"""
import json as _json

import ml_dtypes
import numpy as np

import concourse.bass as bass
import concourse.mybir as mybir
from concourse.bass import ts
from concourse.bass_utils import run_bass_kernel_spmd
from concourse.masks import make_identity
from concourse.tile import TileContext
from concourse.vector_clock import ScopedClock

N_CORES = 8
B, S, DIN, DOUT = 4, 4096, 1024, 1024
POOL, R, TOPK, NUM_TASKS = 5, 8, 3, 5
T_CORE = (B * S) // N_CORES          # 2048 tokens per core
NT = T_CORE // 128                   # 16 tiles of 128 tokens
KC = DIN // 128                      # 8 contraction chunks
R5 = POOL * R                        # 40 concatenated lora rows
BF16 = ml_dtypes.bfloat16

# ---------------------------------------------------------------------------
# Workarounds for this walrus build: at most ONE sync wait per instruction
# (zero on DmaTransposeAnt).  Excess waits are hoisted onto standalone
# EventSemaphore instructions; the Tile exit drain gets its waits emitted as
# separate wait_ge ops.
# ---------------------------------------------------------------------------

_ZERO_WAIT_OPS = {"DmaTransposeAnt"}


def _fixup_bir(bir_bytes):
    bir = _json.loads(bir_bytes)
    n = 0
    for f in bir["functions"]:
        for blk in f["blocks"]:
            out = []
            for inst in blk["instructions"]:
                si = inst.get("sync_info")
                waits = (si or {}).get("on_wait") or []
                cap = 0 if inst.get("opcode") in _ZERO_WAIT_OPS else 1
                if len(waits) > cap:
                    for w in waits[cap:]:
                        n += 1
                        out.append({
                            "debug": inst.get("debug", 0),
                            "engine": inst["engine"],
                            "ins": [], "outs": [],
                            "name": f"{inst['name']}-xw{n}",
                            "opcode": "EventSemaphore",
                            "sync_info": {"on_update": [], "on_wait": [w]},
                        })
                    si["on_wait"] = waits[:cap]
                out.append(inst)
            blk["instructions"] = out
    return _json.dumps(bir).encode()


def _install_fixup(nc):
    orig = nc.to_json_bytes
    nc.to_json_bytes = lambda: _fixup_bir(orig())
    return nc


class _TC(TileContext):
    def _drain_and_barrier(self, tick_clock, wait_clock):
        probe = self.nc.sync.drain()
        wait_clock.add_sem_waits(probe.ins, ScopedClock({None: tick_clock.global_clock}))
        waits = [(w.ant_name, w.wait_value) for w in probe.ins.sync_info.on_wait]
        probe.ins.sync_info.on_wait = []
        name2sem = {v.name: v for v in self.sems.allocated().values()}
        for nm, val in waits:
            self.nc.sync.wait_ge(name2sem[nm], val)
        self.nc.sync.drain()
        self.nc.all_engine_barrier()
        popped = self.nc._tile_sem_poison_stack.pop()
        assert popped is self._sem_poison
        self.nc.clear_and_free_semaphores(list(self.sems.allocated().values()))
        self.nc.all_engine_barrier()


# ---------------------------------------------------------------------------
# Kernel A: partial token-sum  s[1, 1024] = sum_t x[t, :]  (bf16 PE reduce)
# ---------------------------------------------------------------------------

def _build_kernel_a():
    """Fused launch 1: reads x once. Produces the non-delta half of y
    (y = x @ W.T for tiles 0..7), the token-sum s over ALL 16 tiles, and
    stashes the delta tiles' PE-transposed x (bf16) to DRAM for launch 2."""
    nc = bass.Bass(num_devices=N_CORES)
    x_d = nc.dram_tensor("x", [T_CORE, DIN], mybir.dt.float32, kind="ExternalInput")
    wt_d = nc.dram_tensor("wt", [128, KC, DOUT], mybir.dt.bfloat16, kind="ExternalInput")
    yh_d = nc.dram_tensor("yh", [T_CORE // 2, DOUT], mybir.dt.bfloat16, kind="ExternalOutput")
    s_d = nc.dram_tensor("s", [128, KC], mybir.dt.float32, kind="ExternalOutput")
    xts_d = nc.dram_tensor("xts", [NT // 2, 128, KC, 128], mybir.dt.bfloat16,
                           kind="ExternalOutput")
    with _TC(nc) as tc:
        with (tc.tile_pool(name="cst", bufs=1) as cpool,
              tc.tile_pool(name="io", bufs=4) as io,
              tc.tile_pool(name="ys", bufs=3) as yo,
              tc.tile_pool(name="tp", bufs=2, space="PSUM") as tps,
              tc.tile_pool(name="ps", bufs=3, space="PSUM") as ps):
            xb_pre = []
            for i in range(2):
                xb = io.tile([128, DIN], mybir.dt.bfloat16, tag="xb")
                nc.gpsimd.dma_start(out=xb[:], in_=x_d[ts(i, 128), :])
                xb_pre.append(xb)
            ident = cpool.tile([128, 128], mybir.dt.bfloat16)
            make_identity(nc, ident[:])
            wt = cpool.tile([128, KC, DOUT], mybir.dt.bfloat16)
            for h in range(4):
                nc.sync.dma_start(out=wt[:, ts(h, 2), :], in_=wt_d[:, ts(h, 2), :])
            s_cols = cpool.tile([128, NT, KC], mybir.dt.float32)

            for i in range(NT):
                held = i >= NT // 2
                if i < 2:
                    xb = xb_pre[i]
                else:
                    xb = io.tile([128, DIN], mybir.dt.bfloat16, tag="xb")
                    nc.gpsimd.dma_start(out=xb[:], in_=x_d[ts(i, 128), :])
                tp = tps.tile([128, KC, 128], mybir.dt.bfloat16, tag="tp")
                for j in range(KC):
                    nc.tensor.matmul(tp[:, j, :], xb[:, ts(j, 128)], ident[:],
                                     is_transpose=True, start=True, stop=True)
                xt = io.tile([128, KC, 128], mybir.dt.bfloat16, tag="xt")
                nc.vector.tensor_copy(xt[:], tp[:])
                # token-sum contribution: reduce over tokens (innermost axis)
                nc.vector.tensor_reduce(s_cols[:, i, :], xt[:],
                                        mybir.AxisListType.X, mybir.AluOpType.add)
                if held:
                    # stash transposed tile for launch 2 (bf16, 256 KB)
                    nc.sync.dma_start(out=xts_d[i - NT // 2], in_=xt[:])
                else:
                    y0 = ps.tile([128, 512], mybir.dt.float32, tag="y0")
                    y1 = ps.tile([128, 512], mybir.dt.float32, tag="y1")
                    for j in range(KC):
                        nc.tensor.matmul(y0[:], xt[:, j, :], wt[:, j, 0:512],
                                         start=(j == 0), stop=(j == KC - 1))
                        nc.tensor.matmul(y1[:], xt[:, j, :], wt[:, j, 512:1024],
                                         start=(j == 0), stop=(j == KC - 1))
                    ysb = yo.tile([128, DOUT], mybir.dt.bfloat16, tag="ysb")
                    nc.scalar.activation(ysb[:, 0:512], y0[:],
                                         mybir.ActivationFunctionType.Copy)
                    nc.scalar.activation(ysb[:, 512:1024], y1[:],
                                         mybir.ActivationFunctionType.Copy)
                    nc.sync.dma_start(out=yh_d[ts(i, 128), :], in_=ysb[:])
            ss = io.tile([128, KC], mybir.dt.float32, tag="ss")
            nc.vector.tensor_reduce(ss[:], s_cols[:].rearrange("p t c -> p c t"),
                                    mybir.AxisListType.X, mybir.AluOpType.add)
            nc.sync.dma_start(out=s_d[:, :], in_=ss[:])
    return _install_fixup(nc)


# ---------------------------------------------------------------------------
# Kernel B: y = x @ W.T (+ low-rank gated delta on the second-half tiles)
# ---------------------------------------------------------------------------

def _build_kernel_b():
    """Launch 2: delta-half y only, from the stashed transposed x (bf16)
    and the host-folded wmod = W.T + gated rank-40 lora product."""
    nc = bass.Bass(num_devices=N_CORES)
    xts_d = nc.dram_tensor("xts", [NT // 2, 128, KC, 128], mybir.dt.bfloat16,
                           kind="ExternalInput")
    wm_d = nc.dram_tensor("wm", [128, KC, DOUT], mybir.dt.bfloat16, kind="ExternalInput")
    y_d = nc.dram_tensor("y", [T_CORE // 2, DOUT], mybir.dt.bfloat16, kind="ExternalOutput")

    with _TC(nc) as tc:
        with (tc.tile_pool(name="cst", bufs=1) as cpool,
              tc.tile_pool(name="io", bufs=4) as io,
              tc.tile_pool(name="ys", bufs=3) as yo,
              tc.tile_pool(name="ps", bufs=3, space="PSUM") as ps):
            wm = cpool.tile([128, KC, DOUT], mybir.dt.bfloat16)
            nc.sync.dma_start(out=wm[:, 0, :], in_=wm_d[:, 0, :])
            xt_pre = []
            for i in range(2):
                xt = io.tile([128, KC, 128], mybir.dt.bfloat16, tag="xt")
                nc.sync.dma_start(out=xt[:], in_=xts_d[i])
                xt_pre.append(xt)
            for h in range(1, KC):  # chunk-granular so early chunks land first
                nc.sync.dma_start(out=wm[:, h, :], in_=wm_d[:, h, :])

            for i in range(NT // 2):
                if i < 2:
                    xt = xt_pre[i]
                else:
                    xt = io.tile([128, KC, 128], mybir.dt.bfloat16, tag="xt")
                    nc.sync.dma_start(out=xt[:], in_=xts_d[i])
                y0 = ps.tile([128, 512], mybir.dt.float32, tag="y0")
                y1 = ps.tile([128, 512], mybir.dt.float32, tag="y1")
                for j in range(KC):
                    last = j == KC - 1
                    nc.tensor.matmul(y0[:], xt[:, j, :], wm[:, j, 0:512],
                                     start=(j == 0), stop=last)
                    nc.tensor.matmul(y1[:], xt[:, j, :], wm[:, j, 512:1024],
                                     start=(j == 0), stop=last)
                ysb = yo.tile([128, DOUT], mybir.dt.bfloat16, tag="ysb")
                nc.scalar.activation(ysb[:, 0:512], y0[:],
                                     mybir.ActivationFunctionType.Copy)
                nc.scalar.activation(ysb[:, 512:1024], y1[:],
                                     mybir.ActivationFunctionType.Copy)
                nc.sync.dma_start(out=y_d[ts(i, 128), :], in_=ysb[:])
    return _install_fixup(nc)


_NC_CACHE = {}


def _get_nc(name):
    if name not in _NC_CACHE:
        _NC_CACHE[name] = _build_kernel_a() if name == "a" else _build_kernel_b()
    return _NC_CACHE[name]


LAST_RESULTS = {}  # test-harness hook: BassKernelResults of the last call


def kernel(input, W, lora_down, lora_up, lora_route, task_id):
    x = np.ascontiguousarray(np.asarray(input, dtype=np.float32)).reshape(B * S, DIN)
    W = np.asarray(W, dtype=np.float32)
    lora_down = np.asarray(lora_down, dtype=np.float32)
    lora_up = np.asarray(lora_up, dtype=np.float32)
    lora_route = np.asarray(lora_route, dtype=np.float32)
    tid = min(int(task_id), NUM_TASKS)
    k = min(tid, TOPK)

    half = (B * S) // 2
    per = half // N_CORES  # 1024 tokens from each half per core
    shards = [np.concatenate([x[c * per:(c + 1) * per],
                              x[half + c * per:half + (c + 1) * per]])
              for c in range(N_CORES)]
    core_ids = list(range(N_CORES))

    wfull = W.T
    wt_h = np.ascontiguousarray(wfull.reshape(KC, 128, DOUT).transpose(1, 0, 2)).astype(BF16)

    # ---- launch 1: non-delta y + token sums + transposed delta-x stash ----
    res_a = run_bass_kernel_spmd(_get_nc("a"),
                                 [{"x": s, "wt": wt_h} for s in shards], core_ids)
    LAST_RESULTS["a"] = res_a
    s_tot = np.sum([r["s"] for r in res_a.results], axis=0).T.ravel()

    # ---- host gating (5 floats; replicates reference incl. its direct-index
    #      use of top-k positions into the expert pool) ----
    omega = (s_tot / float(B * S)) @ lora_route[1]          # [POOL]
    sliced = omega[1:tid + 1]
    idx = np.argsort(-sliced, kind="stable")[:k]            # top-k positions
    g = np.exp(sliced[idx] - sliced[idx].max())
    gate = g / g.sum()
    w5 = np.zeros(POOL, np.float32)
    for gi, ei in zip(gate, idx):
        w5[ei] += gi                                        # positions used as expert ids
    wrep = np.repeat(w5, R).astype(np.float32)              # [40]

    # fold the gated low-rank delta into a modified weight matrix for the
    # delta-half tiles: wmod = W.T + (down_cat * wrep) @ up_cat
    down_cat = lora_down.transpose(1, 0, 2).reshape(DIN, R5)
    up_cat = lora_up.reshape(R5, DOUT)
    wmod = wfull + (down_cat * wrep[None, :]) @ up_cat
    wm_h = np.ascontiguousarray(wmod.reshape(KC, 128, DOUT).transpose(1, 0, 2)).astype(BF16)

    # ---- launch 2: delta-half y from the stashed transposed x ----
    in_maps = [{"xts": res_a.results[c]["xts"], "wm": wm_h}
               for c in range(N_CORES)]
    res_b = run_bass_kernel_spmd(_get_nc("b"), in_maps, core_ids)
    LAST_RESULTS["b"] = res_b

    y = np.empty((B * S, DOUT), np.float32)
    for c in range(N_CORES):
        y[c * per:(c + 1) * per] = res_a.results[c]["yh"]
        y[half + c * per:half + (c + 1) * per] = res_b.results[c]["y"]
    return y.reshape(B, S, DOUT)

